# revision 17
# baseline (speedup 1.0000x reference)
"""Trainium2 Bass kernel for a DoReFa-quantized ResNet BasicBlock.

    out = act(bn2(conv3x3(act(bn1(conv3x3(x, qw(w1)))), qw(w2))) + x)

with 4-bit DoReFa weight/activation quantization and training-mode BatchNorm
(batch statistics over N,H,W).

Strategy (8 NeuronCores, data-parallel over batch, synced BN):
 - batch N=64 sharded 8 images/core; weights replicated.
 - conv3x3 = shifted matmuls accumulated in PSUM (C_in on partitions,
   pixels free), tap-major over a zero-padded SBUF image so one weight
   load serves 7 chunk matmuls; all matmuls fp8e4m3 DoubleRow (2 taps
   per matmul, 0.5 cycles/output).
 - conv1 input precision: x = a + (b64 + c64)/64 with a=fp8(x),
   b64=fp8(64(x-a)), c64=fp8(64(x-a)-b64).  Taps over the b/c planes use
   weights pre-scaled by 2^-6 (exact in fp8 for odd ints <= 15), giving
   ~16-bit effective input precision at fp8 DoubleRow speed.
 - conv2 is exact: act1 levels are small ints in fp8, weights are odd
   ints (2m-15); the 9th tap rides a DoubleRow pair against zero weights.
 - BN stats: one bn_stats per image ([C,7,448] records) from the SBUF
   copy, bn_aggr'd to per-core sums and AllGather+local-reduced across
   cores; the 1/15, 1/225 conv scales fold into the BN affine vectors.
 - Activation quantization via the 2^23 magic-constant RTNE trick; the
   BN bias folds into the magic constant, clips fuse into one
   tensor_scalar (round commutes with min/max on this grid).  The final
   output is written as integer levels in bf16 (exact) and divided by 15
   on the host.
 - Elementwise work is split across ACT/DVE/Pool by a static balance;
   the residual x is re-read f32 from HBM, prefetched under the stats
   collective window.
"""

import numpy as np

import bass_rust
import concourse.bacc as bacc
import concourse.mybir as mybir
import concourse.bass_isa as bass_isa
import concourse.tile as tile
from concourse.bass_utils import run_bass_kernel_spmd
from concourse.bass_interp import get_hw_module
from concourse.masks import make_identity

F32 = mybir.dt.float32
BF16 = mybir.dt.bfloat16
FP8 = mybir.dt.float8e4
AF = mybir.ActivationFunctionType
ALU = mybir.AluOpType
DR = mybir.MatmulPerfMode.DoubleRow

N_CORES = 8
N_PER = 8            # images per core
C = 128              # channels
H = W = 56
HW = H * W           # 3136
HH = H // 2          # half-image rows
HHW = HW // 2        # half-image pixels (1568)
PW = 58              # padded width
PH = 66              # padded height + 8-row zero block (zero-pair windows)
RCH = 8              # output rows per chunk
NCHUNK = H // RCH    # 7 chunks per image
CHN = RCH * W        # 448 pixels per chunk
NREC = N_PER * NCHUNK
MAGIC = float(2.0 ** 23)
N_SHARD = float(N_PER * HW)           # per-core BN sample count
N_BATCH = 64 * HW                     # full-batch BN sample count
INV_N = float(np.float32(1.0 / N_BATCH))
EPS1 = float(np.float32(225e-5))      # 15^2 * 1e-5   (conv1 output scale)
EPS2 = float(np.float32(50625e-5))    # 225^2 * 1e-5  (conv2 output scale)
SPLIT_S = 64.0                        # residual-split scale (2^6)
INV_SPLIT = float(np.float32(1.0 / SPLIT_S))
# tanh(w) ~ w * (1 + w2*(c1 + w2*(c2 + w2*c3))), |w| < 0.27
TC1 = float(np.float32(-1.0 / 3.0))
TC2 = float(np.float32(2.0 / 15.0))
TC3 = float(np.float32(-17.0 / 315.0))
TAPS = [(ky, kx) for ky in range(3) for kx in range(3)]
TAP_OFF = [ky * PW + kx for ky, kx in TAPS]
NPAIR1 = 14          # conv1 DR pairs (27 blocks + zero-window partner)
NPAIR2 = 5           # conv2 DR pairs (9 taps + zero-window partner)

_CACHED = {}


def _block1(b):
    """conv1 block index -> (split plane s, tap t); s-major ordering keeps
    all pair deltas positive."""
    return b // 9, b % 9


def _pair_rhs1(xpad_ap, r0, p):
    """[C,2,RCH,W] AP for conv1 DoubleRow pair p: two shifted windows of the
    [C,3,PH,PW] split-plane image; dim1 steps between the paired blocks.
    The lone 27th block pairs against the all-zero rows at row PW."""
    b0 = 2 * p
    s0, t0 = _block1(b0)
    ky0, kx0 = TAPS[t0]
    if p < NPAIR1 - 1:
        s1, t1 = _block1(b0 + 1)
        delta = (s1 - s0) * PH * PW + (TAP_OFF[t1] - TAP_OFF[t0])
    else:
        delta = (PW - (r0 + ky0)) * PW - kx0
    base = xpad_ap[:, s0, r0 + ky0:r0 + ky0 + RCH, kx0:kx0 + W]
    u = base.unsqueeze(1).broadcast_to((C, 2, RCH, W)).copy()
    pairs = [tuple(x) for x in u.ap]
    pairs[1] = (delta, 2)
    u.ap = bass_rust.VecI64Pair(pairs)
    return u


def _pair_rhs2(apad_ap, r0, p):
    """[C,2,RCH,W] AP for conv2 DoubleRow pair p over the fp8 act image."""
    t0 = 2 * p
    ky0, kx0 = TAPS[t0]
    delta = (TAP_OFF[t0 + 1] - TAP_OFF[t0]) if p < NPAIR2 - 1 \
        else (PW - (r0 + ky0)) * PW - kx0
    base = apad_ap[:, r0 + ky0:r0 + ky0 + RCH, kx0:kx0 + W]
    u = base.unsqueeze(1).broadcast_to((C, 2, RCH, W)).copy()
    pairs = [tuple(x) for x in u.ap]
    pairs[1] = (delta, 2)
    u.ap = bass_rust.VecI64Pair(pairs)
    return u


def _tanh_poly_multi(nc, parts):
    """wt = taylor_tanh(w) elementwise over several (out, tmp, w2, w) slice
    groups, ops interleaved across groups to hide DVE inter-op latency."""
    steps = [
        lambda o, t, w2, w: nc.vector.tensor_tensor(w2, w, w, ALU.mult),
        lambda o, t, w2, w: nc.vector.tensor_scalar(t, w2, TC3, TC2,
                                                    ALU.mult, ALU.add),
        lambda o, t, w2, w: nc.vector.tensor_tensor(t, t, w2, ALU.mult),
        lambda o, t, w2, w: nc.vector.tensor_scalar(t, t, TC1, None, ALU.add),
        lambda o, t, w2, w: nc.vector.tensor_tensor(t, t, w2, ALU.mult),
        lambda o, t, w2, w: nc.vector.tensor_tensor(o, w, t, ALU.mult),
        lambda o, t, w2, w: nc.vector.tensor_tensor(o, w, o, ALU.add),
    ]
    for step in steps:
        for grp in parts:
            step(*grp)


def _tanh_poly(nc, tt_out, ts_tmp, w2src, wsrc):
    _tanh_poly_multi(nc, [(tt_out, ts_tmp, w2src, wsrc)])


def _quant_weights(nc, pool_T, consts, ptr, ident, w_ap, copy_taps, name):
    """DoReFa-quantize a [128,128,3,3] weight; per-tap transposed integer
    (2m-15) tiles are delivered via copy_taps(tap_index, psum_bf16_ap)."""
    wsb = pool_T.tile([C, C * 9], F32, tag="T")
    nc.sync.dma_start(wsb[:], w_ap.rearrange("o i kh kw -> o (i kh kw)"))
    amax = consts.tile([C, 1], F32, tag=f"amax{name}")
    nc.vector.tensor_reduce(amax[:], wsb[:], mybir.AxisListType.X, ALU.max,
                            apply_absolute_value=True)
    gmax = consts.tile([C, 1], F32, tag=f"gmax{name}")
    nc.gpsimd.partition_all_reduce(gmax[:], amax[:], C, bass_isa.ReduceOp.max)
    mt1 = consts.tile([C, 1], F32, tag=f"mt1{name}")
    mt2 = consts.tile([C, 1], F32, tag=f"mt2{name}")
    mval = consts.tile([C, 1], F32, tag=f"mval{name}")
    _tanh_poly(nc, mval[:], mt1[:], mt2[:], gmax[:])
    # s15 = 15 / (2*M); wn15 = wt*s15 + 7.5; codes = rtne(wn15)
    inv2m = consts.tile([C, 1], F32, tag=f"inv2m{name}")
    nc.vector.tensor_scalar(inv2m[:], mval[:], 2.0, None, ALU.mult)
    nc.vector.reciprocal(inv2m[:], inv2m[:])
    s15 = consts.tile([C, 1], F32, tag=f"s15{name}")
    nc.vector.tensor_scalar(s15[:], inv2m[:], 15.0, None, ALU.mult)
    w2t = pool_T.tile([C, C * 9], F32, tag="T")
    qt = pool_T.tile([C, C * 9], F32, tag="T")
    wt = pool_T.tile([C, C * 9], F32, tag="T")
    wn = pool_T.tile([C, C * 9], F32, tag="T")
    wi = pool_T.tile([C, C * 9], BF16, tag="T")
    HC = C * 9 // 2
    halves = [slice(0, HC), slice(HC, C * 9)]
    _tanh_poly_multi(nc, [(wt[:, hs], qt[:, hs], w2t[:, hs], wsb[:, hs])
                          for hs in halves])
    for hs in halves:
        nc.vector.tensor_scalar(wn[:, hs], wt[:, hs], s15[:, 0:1], 7.5,
                                ALU.mult, ALU.add)
    for hs in halves:
        nc.vector.tensor_scalar(wn[:, hs], wn[:, hs], MAGIC, -MAGIC,
                                ALU.add, ALU.add)
    for hs in halves:
        nc.vector.tensor_scalar(wi[:, hs], wn[:, hs], 2.0, -15.0,
                                ALU.mult, ALU.add)
    # transpose each tap via PE: lhsT[i, o] = Wi[o, i*9+t]
    wir = wi.rearrange("o (i t) -> o i t", t=9)
    for t in range(9):
        pst = ptr.tile([C, C], BF16, tag="tr")
        nc.tensor.transpose(pst[:], wir[:, :, t], ident[:])
        copy_taps(t, pst)


def _stats_to_sums(nc, statsp, stats_rec, name):
    """bn_aggr the [C, NREC, 6] records into per-core [sum, sumsq]."""
    mv = statsp.tile([C, 2], F32, tag=f"mv{name}")
    nc.vector.bn_aggr(mv[:], stats_rec)
    msq = statsp.tile([C, 1], F32, tag=f"msq{name}")
    nc.vector.tensor_tensor(msq[:], mv[:, 0:1], mv[:, 0:1], ALU.mult)
    st = statsp.tile([C, 2], F32, tag=f"st{name}")
    nc.vector.tensor_scalar(st[:, 0:1], mv[:, 0:1], N_SHARD, None, ALU.mult)
    nc.vector.tensor_tensor(msq[:], mv[:, 1:2], msq[:], ALU.add)
    nc.vector.tensor_scalar(st[:, 1:2], msq[:], N_SHARD, None, ALU.mult)
    return st


def _ag_sum(nc, statsp, dram, st, RG, name):
    """Cross-core sum of a [C,2] stats tile via AllGather + local reduce
    (AG has a lower latency floor than AllReduce for tiny payloads)."""
    agi = dram.tile([C, 2], F32, tag=f"agi{name}")
    ago = dram.tile([N_CORES, C, 2], F32, tag=f"ago{name}")
    nc.gpsimd.dma_start(agi[:], st[:])
    nc.gpsimd.collective_compute(
        "AllGather", ALU.bypass, replica_groups=RG,
        ins=[agi.opt()], outs=[ago.opt()])
    allst = statsp.tile([C, 2, N_CORES], F32, tag=f"allst{name}")
    nc.gpsimd.dma_start(allst[:], ago.rearrange("r c s -> c s r"))
    rst = statsp.tile([C, 2], F32, tag=f"rst{name}")
    nc.vector.tensor_reduce(rst[:], allst[:], mybir.AxisListType.X, ALU.add)
    return rst


def _bn_vectors(nc, consts, rstats, gamma_ap, beta_ap, eps, post_scale, name):
    """Per-channel scale/bias [128,1] s.t. T*scale + bias equals
    post_scale * batchnorm(T/k); eps is pre-scaled by k^2."""
    g = consts.tile([C, 1], F32, tag=f"g{name}")
    nc.sync.dma_start(g[:], gamma_ap.rearrange("(c one) -> c one", one=1))
    b = consts.tile([C, 1], F32, tag=f"b{name}")
    nc.sync.dma_start(b[:], beta_ap.rearrange("(c one) -> c one", one=1))
    mean = consts.tile([C, 1], F32, tag=f"mean{name}")
    nc.vector.tensor_scalar(mean[:], rstats[:, 0:1], INV_N, None, ALU.mult)
    var = consts.tile([C, 1], F32, tag=f"var{name}")
    nc.vector.tensor_scalar(var[:], rstats[:, 1:2], INV_N, None, ALU.mult)
    msq = consts.tile([C, 1], F32, tag=f"msq{name}")
    nc.vector.tensor_tensor(msq[:], mean[:], mean[:], ALU.mult)
    nc.vector.tensor_tensor(var[:], var[:], msq[:], ALU.subtract)
    epst = consts.tile([C, 1], F32, tag=f"eps{name}")
    nc.vector.memset(epst[:], eps)
    std = consts.tile([C, 1], F32, tag=f"std{name}")
    nc.scalar.activation(std[:], var[:], AF.Sqrt, bias=epst[:, 0:1], scale=1.0)
    inv = consts.tile([C, 1], F32, tag=f"inv{name}")
    nc.vector.reciprocal(inv[:], std[:])
    scale = consts.tile([C, 1], F32, tag=f"scale{name}")
    nc.vector.tensor_tensor(scale[:], g[:], inv[:], ALU.mult)
    nc.vector.tensor_scalar(scale[:], scale[:], post_scale, None, ALU.mult)
    bias = consts.tile([C, 1], F32, tag=f"bias{name}")
    nc.vector.tensor_tensor(bias[:], mean[:], scale[:], ALU.mult)
    nc.vector.tensor_scalar(b[:], b[:], post_scale, None, ALU.mult)
    nc.vector.tensor_tensor(bias[:], b[:], bias[:], ALU.subtract)
    return scale, bias


def build():
    nc = bacc.Bacc("TRN2", target_bir_lowering=False, debug=False,
                   num_devices=N_CORES)
    x_ap = nc.dram_tensor("x", [N_PER, C, H, W], F32, kind="ExternalInput").ap()
    w1_ap = nc.dram_tensor("w1", [C, C, 3, 3], F32, kind="ExternalInput").ap()
    w2_ap = nc.dram_tensor("w2", [C, C, 3, 3], F32, kind="ExternalInput").ap()
    g1_ap = nc.dram_tensor("gamma1", [C], F32, kind="ExternalInput").ap()
    b1_ap = nc.dram_tensor("beta1", [C], F32, kind="ExternalInput").ap()
    g2_ap = nc.dram_tensor("gamma2", [C], F32, kind="ExternalInput").ap()
    b2_ap = nc.dram_tensor("beta2", [C], F32, kind="ExternalInput").ap()
    # integer act levels 0..15; host divides by 15
    out_ap = nc.dram_tensor("out", [N_PER, C, H, W], BF16,
                            kind="ExternalOutput").ap()
    x_r = x_ap.rearrange("n c h w -> n c h w")
    x_f = x_ap.rearrange("n c h w -> n c (h w)")
    out_f = out_ap.rearrange("n c h w -> n c (h w)")
    RG = [list(range(N_CORES))]

    with tile.TileContext(nc) as tc:
        with tc.tile_pool(name="consts", bufs=1) as consts, \
             tc.tile_pool(name="T", bufs=N_PER) as pool_T, \
             tc.tile_pool(name="scratch", bufs=4) as scratch, \
             tc.tile_pool(name="xpad", bufs=2) as xpadp, \
             tc.tile_pool(name="apad", bufs=2) as apadp, \
             tc.tile_pool(name="xres", bufs=2) as xresp, \
             tc.tile_pool(name="outq", bufs=3) as outqp, \
             tc.tile_pool(name="psum", bufs=6, space="PSUM") as psum, \
             tc.tile_pool(name="ptr", bufs=2, space="PSUM") as ptr, \
             tc.tile_pool(name="stats", bufs=1) as statsp, \
             tc.tile_pool(name="dram", bufs=1, space="DRAM") as dram:

            # dummy collective to absorb mesh/barrier warmup during conv1
            warm = statsp.tile([C, 1], F32, tag="warm")
            nc.vector.memset(warm[:], 0.0)
            ccwi = dram.tile([C, 1], F32, tag="ccwi")
            ccwo = dram.tile([C, 1], F32, tag="ccwo")
            nc.gpsimd.dma_start(ccwi[:], warm[:])
            nc.gpsimd.collective_compute(
                "AllReduce", ALU.add, replica_groups=RG,
                ins=[ccwi.opt()], outs=[ccwo.opt()])

            ident = consts.tile([C, C], BF16, tag="ident")
            make_identity(nc, ident[:])
            # conv1: 14 fp8 DR pairs over 27 blocks (3 planes x 9 taps)
            lhsT1p = consts.tile([C, NPAIR1, 2, C], FP8, tag="lhsT1p")
            # conv2: 5 fp8 DR pairs over 9 taps (+1 zero block)
            lhsT2p = consts.tile([C, NPAIR2, 2, C], FP8, tag="lhsT2p")
            def copy1(t, pst):
                for s in range(3):
                    b = s * 9 + t
                    sc = 1.0 if s == 0 else INV_SPLIT
                    nc.scalar.mul(lhsT1p[:, b // 2, b % 2, :], pst[:], sc)
                    if b == 26:  # partner reads the zero rows; weights moot
                        nc.scalar.mul(lhsT1p[:, b // 2, 1, :], pst[:], sc)

            def copy2(t, pst):
                if t < 8:
                    nc.scalar.copy(lhsT2p[:, t // 2, t % 2, :], pst[:])
                else:
                    nc.scalar.copy(lhsT2p[:, NPAIR2 - 1, 0, :], pst[:])
                    nc.scalar.copy(lhsT2p[:, NPAIR2 - 1, 1, :], pst[:])

            def prep_image(i, cnt):
                """Load image i and write its 3 fp8 split planes."""
                xp = xpadp.tile([C, 3, PH, PW], FP8, tag="xpad")
                # zero the pad border + the 8-row zero-window block
                nc.gpsimd.memset(xp[:, :, 0, :], 0.0)
                nc.gpsimd.memset(xp[:, :, PW - 1:PH, :], 0.0)
                nc.gpsimd.memset(xp[:, :, 1:PW - 1, 0:1], 0.0)
                nc.gpsimd.memset(xp[:, :, 1:PW - 1, PW - 1:PW], 0.0)
                for h in range(2):
                    rows = slice(1 + h * HH, 1 + (h + 1) * HH)
                    cols = slice(1, 1 + W)
                    xs = scratch.tile([C, HH, W], F32, tag="sc")
                    nc.sync.dma_start(xs[:], x_r[i, :, h * HH:(h + 1) * HH, :])
                    a = xp[:, 0, rows, cols]
                    nc.scalar.copy(a, xs[:])
                    r = scratch.tile([C, HH, W], F32, tag="sc")
                    # r = x - a; b64 = fp8(64r); c64 = fp8(64r - b64)
                    nc.vector.scalar_tensor_tensor(r[:], a, -1.0, xs[:],
                                                   ALU.mult, ALU.add)
                    nc.scalar.mul(xp[:, 1, rows, cols], r[:], SPLIT_S)
                    nc.vector.scalar_tensor_tensor(
                        xp[:, 2, rows, cols], r[:], SPLIT_S,
                        xp[:, 1, rows, cols], ALU.mult, ALU.subtract)
                return xp

            # prep image 0 (DMA/ACT/DVE/Pool only), then quantize w1 so conv1
            # starts as soon as lhsT1p is ready; w2 quant overlaps conv1.
            cnt = [0]
            prepped = {0: prep_image(0, cnt)}
            with nc.named_scope("wquant1"):
                _quant_weights(nc, pool_T, consts, ptr, ident, w1_ap, copy1, "1")
            prepped[1] = prep_image(1, cnt)
            # wquant2 must precede the conv1 loop: the shared T-pool ring
            # only stays collision-free in this allocation order.  Its DVE
            # chain overlaps conv1 of images 0-1 via the in-order queues.
            with nc.named_scope("wquant2"):
                _quant_weights(nc, pool_T, consts, ptr, ident, w2_ap, copy2, "2")

            # ---------------- phase 1: conv1 + stats ----------------
            stats1 = statsp.tile([C, N_PER, NCHUNK, 6], F32, tag="stats1")
            T1 = []
            with nc.named_scope("conv1"):
                for i in range(N_PER):
                    xp = prepped.pop(i) if i in prepped else prep_image(i, cnt)
                    Ti = pool_T.tile([C, HW], F32, tag="T")
                    T1.append(Ti)
                    Tir = Ti.rearrange("c (h w) -> c h w", w=W)
                    pss = []
                    for ck in range(NCHUNK):
                        ps_t = psum.tile([C, CHN], F32, tag="mm",
                                         name=f"ps1_{i}_{ck}")
                        pss.append(ps_t)
                        for p in range(NPAIR1):
                            nc.tensor.matmul(
                                ps_t[:], lhsT1p[:, p, :, :],
                                _pair_rhs1(xp, ck * RCH, p),
                                start=(p == 0), stop=(p == NPAIR1 - 1),
                                perf_mode=DR)
                    if i + 1 < N_PER and (i + 1) not in prepped:
                        prepped[i + 1] = prep_image(i + 1, cnt)
                    for ck in range(NCHUNK):
                        nc.scalar.copy(Tir[:, ck * RCH:(ck + 1) * RCH, :],
                                       pss[ck][:])
                        nc.vector.bn_stats(stats1[:, i, ck, :], pss[ck][:])
                    if i == 1:
                        with nc.named_scope("wquant2"):
                            _quant_weights(nc, pool_T, consts, ptr, ident,
                                           w2_ap, copy2, "2")
                    if i == 6:
                        # mid-flight cross-core sync absorbs skew before AG1
                        ccs = dram.tile([C, 6], F32, tag="ccs")
                        ccso = dram.tile([C, 6], F32, tag="ccso")
                        nc.gpsimd.dma_start(ccs[:], stats1[:, 6, 0, :])
                        nc.gpsimd.collective_compute(
                            "AllReduce", ALU.add, replica_groups=RG,
                            ins=[ccs.opt()], outs=[ccso.opt()])

            # allreduce stats 1 -> act1 affine (y = 15*bn1(T1/15))
            st1 = _stats_to_sums(
                nc, statsp, stats1.rearrange("c n g s -> c (n g) s"), "1")
            rst1 = _ag_sum(nc, statsp, dram, st1, RG, "1")
            sc1, bi1 = _bn_vectors(nc, consts, rst1, g1_ap, b1_ap, EPS1,
                                   15.0, "1")

            nmag1 = consts.tile([C, 1], F32, tag="nmag1")
            nc.vector.memset(nmag1[:], -MAGIC)

            # ---------------- phase 2: act1 + conv2 + stats ----------------
            stats2 = statsp.tile([C, N_PER, NCHUNK, 6], F32, tag="stats2")
            T2 = []

            def act1_image(i):
                ap_t = apadp.tile([C, PH, PW], FP8, tag="apad",
                                  name=f"apad{i}")
                nc.gpsimd.memset(ap_t[:, 0, :], 0.0)
                nc.gpsimd.memset(ap_t[:, PW - 1:PH, :], 0.0)
                nc.gpsimd.memset(ap_t[:, 1:PW - 1, 0:1], 0.0)
                nc.gpsimd.memset(ap_t[:, 1:PW - 1, PW - 1:PW], 0.0)
                T1r = T1[i].rearrange("c (h w) -> c h w", w=W)
                for h in range(2):
                    rows = slice(h * HH, (h + 1) * HH)
                    prow = slice(1 + h * HH, 1 + (h + 1) * HH)
                    y = scratch.tile([C, HH, W], F32, tag="sc",
                                     name=f"y{i}_{h}")
                    nc.scalar.activation(y[:], T1r[:, rows, :], AF.Relu,
                                         bias=bi1[:, 0:1], scale=sc1[:, 0:1])
                    # round (+high clip); low clip already via Relu
                    nc.vector.tensor_scalar(y[:], y[:], MAGIC, MAGIC + 15.0,
                                            ALU.add, ALU.min)
                    nc.scalar.activation(ap_t[:, prow, 1:1 + W], y[:],
                                         AF.Identity, bias=nmag1[:, 0:1],
                                         scale=1.0)
                return ap_t

            with nc.named_scope("act1_conv2"):
                apads = {0: act1_image(0)}
                for i in range(N_PER):
                    ap_t = apads.pop(i)
                    Ti2 = pool_T.tile([C, HW], F32, tag="T")
                    T2.append(Ti2)
                    T2r = Ti2.rearrange("c (h w) -> c h w", w=W)
                    pss = []
                    for ck in range(NCHUNK):
                        ps_t = psum.tile([C, CHN], F32, tag="mm",
                                         name=f"ps2_{i}_{ck}")
                        pss.append(ps_t)
                        for p in range(NPAIR2):
                            nc.tensor.matmul(
                                ps_t[:], lhsT2p[:, p, :, :],
                                _pair_rhs2(ap_t, ck * RCH, p),
                                start=(p == 0), stop=(p == NPAIR2 - 1),
                                perf_mode=DR)
                    if i + 1 < N_PER:
                        apads[i + 1] = act1_image(i + 1)
                    for ck in range(NCHUNK):
                        eng = nc.scalar.copy if ck < 4 else (
                            lambda o_, i_: nc.vector.tensor_scalar(
                                o_, i_, 0.0, None, ALU.add))
                        eng(T2r[:, ck * RCH:(ck + 1) * RCH, :], pss[ck][:])
                        nc.vector.bn_stats(stats2[:, i, ck, :], pss[ck][:])

            # allreduce stats 2 -> bn2 affine (z = bn2(T2/225))
            st2 = _stats_to_sums(
                nc, statsp, stats2.rearrange("c n g s -> c (n g) s"), "2")
            rst2 = _ag_sum(nc, statsp, dram, st2, RG, "2")
            sc2, bi2 = _bn_vectors(nc, consts, rst2, g2_ap, b2_ap, EPS2,
                                   1.0, "2")
            # k = relu(min(rtne(15*sc2*T2 + 15*x + 15*bi2), 15)); the
            # per-channel factors ride ACT (bias) and DVE (stt scalar) since
            # Pool only accepts immediate scalars.
            nmag = consts.tile([C, 1], F32, tag="nmag")
            nc.vector.memset(nmag[:], -MAGIC)
            sc215 = consts.tile([C, 1], F32, tag="sc215")
            nc.vector.tensor_scalar(sc215[:], sc2[:], 15.0, None, ALU.mult)
            bi215 = consts.tile([C, 1], F32, tag="bi215")
            nc.vector.tensor_scalar(bi215[:], bi2[:], 15.0, None, ALU.mult)

            # ------------- phase 3: bn2 + residual + act -> out -------------
            with nc.named_scope("final"):
                for i in range(N_PER):
                    xr = xresp.tile([C, HW], F32, tag="xres")
                    nc.sync.dma_start(xr[:], x_f[i, :, :])
                    xr2 = xr.rearrange("c (h f) -> c h f", h=2)
                    T22 = T2[i].rearrange("c (h f) -> c h f", h=2)
                    oq = outqp.tile([C, HW], BF16, tag="outq")
                    oq2 = oq.rearrange("c (h f) -> c h f", h=2)
                    for h in range(2):
                        xb = scratch.tile([C, HH, W], F32, tag="sc")
                        xbf = xb.rearrange("c h w -> c (h w)")
                        nc.scalar.activation(xbf, xr2[:, h, :], AF.Identity,
                                             bias=bi215[:, 0:1], scale=15.0)
                        nc.vector.scalar_tensor_tensor(
                            xbf, T22[:, h, :], sc215[:, 0:1], xbf,
                            ALU.mult, ALU.add)
                        nc.vector.tensor_scalar(xbf, xbf, MAGIC, MAGIC + 15.0,
                                                ALU.add, ALU.min)
                        nc.scalar.activation(oq2[:, h, :], xbf, AF.Relu,
                                             bias=nmag[:, 0:1], scale=1.0)
                    nc.sync.dma_start(out_f[i, :, :], oq[:])

    nc.compile()
    return nc


def kernel(x, w1, w2, gamma1, beta1, gamma2, beta2):
    if "nc" not in _CACHED:
        _CACHED["nc"] = build()
    nc = _CACHED["nc"]
    x = np.ascontiguousarray(x, dtype=np.float32)
    shard = x.reshape(N_CORES, N_PER, C, H, W)
    common = {
        "w1": np.ascontiguousarray(w1, np.float32),
        "w2": np.ascontiguousarray(w2, np.float32),
        "gamma1": np.ascontiguousarray(gamma1, np.float32),
        "beta1": np.ascontiguousarray(beta1, np.float32),
        "gamma2": np.ascontiguousarray(gamma2, np.float32),
        "beta2": np.ascontiguousarray(beta2, np.float32),
    }
    in_maps = [{"x": shard[i], **common} for i in range(N_CORES)]
    old_m = nc.m
    nc.m = get_hw_module(nc.m)
    try:
        res = run_bass_kernel_spmd(nc, in_maps, core_ids=list(range(N_CORES)))
    finally:
        nc.m = old_m
    out = np.concatenate([res.results[i]["out"] for i in range(N_CORES)],
                         axis=0)
    return (out.astype(np.float32) / 15.0).astype(np.float32)
